# revision 28
# baseline (speedup 1.0000x reference)
"""GuidedFilterLayer Trainium2 kernel (8 NeuronCores, batch-sharded).

v5: no collective (per-core mean), bf16 planar layout, host pre-scale by
0.99, rolling-window scan row blur, banded TensorE column blur.
See git-less history: restored baseline for device health check.
"""

import numpy as np

B, H, W, C = 16, 512, 512, 3
NCORES = 8
B_LOC = B // NCORES
ROWS = B_LOC * H
FREE = W * C
NCHUNK = ROWS // 128
MPERIM = H // 128
NPIX_LOC = B_LOC * H * W
R_ = 7
K_ = 15
EPS = 0.01
W0, W1, W2 = 0.2989, 0.5870, 0.1140
A1 = 1.0 / W0 - 1.0 / W1
A2 = 1.0 / W1 - 1.0 / W2
A3 = 1.0 / W2
SCALE_SM = EPS / (K_ * K_)
BIAS_SM = -EPS
CMAIN = 1.0 - EPS
NGA_DVE = 3

_cache = {}


def _band_blocks():
    idx = np.arange(2 * 128)
    band = (np.abs(idx[:, None] - idx[None, :]) <= R_).astype(np.float32)
    bdiag = band[0:128, 0:128]
    bup = band[0:128, 128:256]
    bdn = band[128:256, 0:128]
    return np.concatenate([bdiag, bup, bdn], axis=1)


def _wmap_pm():
    i = np.arange(H)
    wr = (np.minimum(i + R_, H - 1) - np.maximum(i - R_, 0) + 1).astype(np.float32)
    wm = wr[:, None] * wr[None, :]
    return np.ascontiguousarray(
        wm.reshape(MPERIM, 128, W).transpose(1, 0, 2).reshape(128, MPERIM * W))


def _build():
    from contextlib import ExitStack
    from concourse import bass, bacc, tile
    import concourse.mybir as mybir
    import ml_dtypes

    f32 = mybir.dt.float32
    bf16 = mybir.dt.bfloat16
    Alu = mybir.AluOpType
    Act = mybir.ActivationFunctionType

    nc = bacc.Bacc(
        "TRN2",
        target_bir_lowering=False,
        debug=False,
        enable_asserts=False,
        num_devices=NCORES,
    )

    x_in = nc.dram_tensor("x", [ROWS, FREE], bf16, kind="ExternalInput")
    out_d = nc.dram_tensor("out", [ROWS, FREE], bf16, kind="ExternalOutput")
    bands_d = nc.inline_tensor(
        _band_blocks().astype(ml_dtypes.bfloat16), name="bands")
    wmap_d = nc.inline_tensor(
        _wmap_pm().astype(ml_dtypes.bfloat16), name="wmap")

    GW = K_ + W + R_               # 534
    SCW = W + R_                   # 519

    with tile.TileContext(nc) as tc, ExitStack() as ctx:
        xp = ctx.enter_context(tc.tile_pool(name="xp", bufs=NCHUNK))
        gp = ctx.enter_context(tc.tile_pool(name="gp", bufs=4))
        gcp = ctx.enter_context(tc.tile_pool(name="gcp", bufs=NCHUNK))
        rbp = ctx.enter_context(tc.tile_pool(name="rbp", bufs=NCHUNK))
        smp = ctx.enter_context(tc.tile_pool(name="smp", bufs=NCHUNK))
        cbp = ctx.enter_context(tc.tile_pool(name="cbp", bufs=3))
        op = ctx.enter_context(tc.tile_pool(name="op", bufs=4))
        cp = ctx.enter_context(tc.tile_pool(name="cp", bufs=1))
        pcb = ctx.enter_context(tc.tile_pool(name="pcb", bufs=2, space="PSUM"))
        prp = ctx.enter_context(tc.tile_pool(name="prp", bufs=1, space="PSUM"))

        xts = []
        for t in range(NCHUNK):
            xt = xp.tile([128, FREE], bf16, tag="x")
            keng = nc.sync if t % 2 == 0 else nc.gpsimd
            keng.dma_start(out=xt[:], in_=x_in[128 * t:128 * (t + 1), :])
            xts.append(xt)

        bsb = cp.tile([128, 384], bf16, tag="bands")
        nc.scalar.dma_start(out=bsb[:], in_=bands_d[:])
        wmt = cp.tile([128, MPERIM, W], bf16, tag="wmt")
        nc.scalar.dma_start(
            out=wmt[:], in_=wmap_d[:].rearrange("p (m w) -> p m w", m=MPERIM))

        gcs = []
        for t in range(NCHUNK):
            g = gcp.tile([128, GW], bf16, tag="gc")
            nc.gpsimd.memset(g[:], 0.0)
            gcs.append(g)
        ones = cp.tile([128, 128], f32, tag="ones")
        nc.gpsimd.memset(ones[:], 1.0)
        zcol = cp.tile([128, 1], bf16, tag="zcol")
        nc.vector.memset(zcol[:], 0.0)

        accs = cp.tile([128, 3 * NCHUNK], f32, tag="accs")
        rbs = [None] * NCHUNK
        sms = [None] * NCHUNK
        d3 = cp.tile([128, 1], f32, tag="d3")
        wmd = cp.tile([128, MPERIM, W], bf16, tag="wmd")

        def gray(t):
            x3 = xts[t][:].rearrange("p (c w) -> p c w", c=C)
            ga = gp.tile([128, W], bf16, tag="ga")
            gb = gp.tile([128, W], bf16, tag="gb")
            if t < NGA_DVE:
                nc.vector.scalar_tensor_tensor(
                    out=ga[:], in0=x3[:, 0, :], scalar=W0 / CMAIN,
                    in1=zcol[:].broadcast_to([128, W]),
                    op0=Alu.mult, op1=Alu.add, accum_out=accs[:, t:t + 1])
            else:
                nc.scalar.activation(
                    out=ga[:], in_=x3[:, 0, :], func=Act.Copy, bias=0.0,
                    scale=W0 / CMAIN, accum_out=accs[:, t:t + 1])
            nc.vector.scalar_tensor_tensor(
                out=gb[:], in0=x3[:, 1, :], scalar=W1 / CMAIN, in1=ga[:],
                op0=Alu.mult, op1=Alu.add,
                accum_out=accs[:, NCHUNK + t:NCHUNK + t + 1])
            nc.vector.scalar_tensor_tensor(
                out=gcs[t][:, K_:K_ + W], in0=x3[:, 2, :], scalar=W2 / CMAIN,
                in1=gb[:], op0=Alu.mult, op1=Alu.add,
                accum_out=accs[:, 2 * NCHUNK + t:2 * NCHUNK + t + 1])

        def rowblur(t):
            rb = rbp.tile([128, SCW], bf16, tag="rb")
            nc.vector.tensor_tensor_scan(
                out=rb[:], data0=gcs[t][:, K_:GW], data1=gcs[t][:, 0:SCW],
                initial=0.0, op0=Alu.add, op1=Alu.subtract)
            rbs[t] = rb

        def colblur(im):
            for mo in range(MPERIM):
                tt = im * MPERIM + mo
                pc = pcb.tile([128, W], f32, tag="pc")
                ks = [(mo, 0)]
                if mo > 0:
                    ks.append((mo - 1, 1))
                if mo < MPERIM - 1:
                    ks.append((mo + 1, 2))
                for j, (kk, blk) in enumerate(ks):
                    nc.tensor.matmul(
                        out=pc[:],
                        lhsT=bsb[:, 128 * blk:128 * (blk + 1)],
                        rhs=rbs[im * MPERIM + kk][:, R_:R_ + W],
                        start=(j == 0), stop=(j == len(ks) - 1))
                sm = smp.tile([128, W], bf16, tag="sm")
                nc.scalar.activation(
                    out=sm[:], in_=pc[:], func=Act.Copy,
                    bias=BIAS_SM, scale=SCALE_SM)
                sms[tt] = sm

        for t in range(MPERIM):
            gray(t)
            rowblur(t)
        for t in range(MPERIM, NCHUNK):
            gray(t)
        colblur(0)

        red3 = cp.tile([128, 4], f32, tag="red3")
        for k in range(3):
            nc.vector.tensor_reduce(
                out=red3[:, k:k + 1], in_=accs[:, k * NCHUNK:(k + 1) * NCHUNK],
                axis=mybir.AxisListType.X, op=Alu.add)
        sb2 = cp.tile([128, 2], f32, tag="sb2")
        tmp = cp.tile([128, 2], f32, tag="tmp")
        nc.vector.tensor_scalar(
            out=tmp[:, 0:1], in0=red3[:, 0:1], scalar1=float(A1), scalar2=None,
            op0=Alu.mult)
        nc.vector.scalar_tensor_tensor(
            out=tmp[:, 1:2], in0=red3[:, 1:2], scalar=float(A2), in1=tmp[:, 0:1],
            op0=Alu.mult, op1=Alu.add)
        nc.vector.scalar_tensor_tensor(
            out=sb2[:, 0:1], in0=red3[:, 2:3], scalar=float(A3), in1=tmp[:, 1:2],
            op0=Alu.mult, op1=Alu.add)
        nc.vector.tensor_copy(out=sb2[:, 1:2], in_=red3[:, 2:3])
        pred = prp.tile([128, 2], f32, tag="pred")
        nc.tensor.matmul(out=pred[:], lhsT=ones[:], rhs=sb2[:],
                         start=True, stop=True)
        redb = cp.tile([128, 2], f32, tag="redb")
        nc.scalar.copy(out=redb[:], in_=pred[:])

        d1 = cp.tile([128, 1], f32, tag="d1")
        d2 = cp.tile([128, 1], f32, tag="d2")
        nc.vector.tensor_scalar(
            out=d1[:], in0=redb[:, 0:1], scalar1=1.0 / (3.0 * NPIX_LOC),
            scalar2=None, op0=Alu.mult)
        nc.vector.scalar_tensor_tensor(
            out=d2[:], in0=redb[:, 1:2], scalar=-1.0 / NPIX_LOC, in1=d1[:],
            op0=Alu.mult, op1=Alu.add)
        nc.vector.tensor_scalar(
            out=d3[:], in0=d2[:], scalar1=1.0, scalar2=float(SCALE_SM),
            op0=Alu.add, op1=Alu.mult)
        for mm in range(MPERIM):
            nc.scalar.activation(
                out=wmd[:, mm, :], in_=wmt[:, mm, :], func=Act.Copy,
                bias=0.0, scale=d3[:])

        for t in range(MPERIM, NCHUNK):
            rowblur(t)
        colblur(1)

        for t in range(NCHUNK):
            im, mm = divmod(t, MPERIM)
            cb = cbp.tile([128, W], bf16, tag="cb")
            nc.vector.tensor_tensor(
                out=cb[:], in0=sms[t][:], in1=wmd[:, mm, :], op=Alu.add)
            ot = op.tile([128, FREE], bf16, tag="o")
            nc.vector.tensor_tensor(
                out=ot[:].rearrange("p (c w) -> p c w", c=C),
                in0=xts[t][:].rearrange("p (c w) -> p c w", c=C),
                in1=cb[:, None, :].broadcast_to([128, C, W]),
                op=Alu.add)
            nc.sync.dma_start(out=out_d[128 * t:128 * (t + 1), :], in_=ot[:])

    nc.finalize()
    return nc


def _get_nc():
    if "nc" not in _cache:
        _cache["nc"] = _build()
    return _cache["nc"]


def _in_maps(x):
    import ml_dtypes

    x = np.asarray(x, dtype=np.float32)
    assert x.shape == (B, H, W, C)
    xs = np.ascontiguousarray(x.transpose(0, 1, 3, 2)) * np.float32(CMAIN)
    xp = xs.astype(ml_dtypes.bfloat16)
    return [
        {"x": np.ascontiguousarray(
            xp[i * B_LOC:(i + 1) * B_LOC].reshape(ROWS, FREE))}
        for i in range(NCORES)
    ]


def _assemble(results):
    out = np.concatenate(
        [np.asarray(results[i]["out"]).reshape(B_LOC, H, C, W)
         for i in range(NCORES)], axis=0)
    return np.ascontiguousarray(out.transpose(0, 1, 3, 2)).astype(np.float32)


def kernel(x):
    from concourse.bass_utils import run_bass_kernel_spmd

    nc = _get_nc()
    res = run_bass_kernel_spmd(nc, _in_maps(x), core_ids=list(range(NCORES)))
    return _assemble(res.results)
